# revision 3
# baseline (speedup 1.0000x reference)
"""Trainium2 Bass kernel for nn_RNN: h_t = x_t @ W + h_{t-1} @ R (linear RNN). v2.

Full shapes: sequences [64, 512, 1024], kernel [1024, 1024],
recurrent_kernel [1024, 1024], h0 [64, 1024] -> out [64, 512, 1024].
Sharding: data-parallel over batch across 8 cores (8 sequences/core).

Per-core blocked scan, K=16 block length, NB=32 blocks, lane r = blk*8+b.
v2 changes vs v1:
  - xproj kept SBUF-resident (bf16, 64KB/partition); no DRAM xp bounce.
  - X transposes via DMA xbar (seq cast to bf16 time-major in DRAM by a
    SWDGE cast DMA, then hardware DMA-transpose loads), not PE.
  - squaring-chain transposes via DRAM bf16 bounce + DMA-transpose, not PE.
  - output path: osc bf16 bounce (ACT queue) -> DMA-transpose (SP queue)
    -> SWDGE cast bf16->fp32 store (Pool queue); no DVE copy.
"""
import sys
import numpy as np

sys.path.insert(0, "/opt/trn_rl_repo")

try:  # persistent jit cache: repeated kernel() invocations skip recompile
    import jax
    import os as _os
    _cache = _os.environ.get("JAX_COMPILATION_CACHE_DIR", "/tmp/jaxcache_rnn")
    _os.makedirs(_cache, exist_ok=True)
    jax.config.update("jax_compilation_cache_dir", _cache)
except Exception:
    pass

import concourse.bass as bass  # noqa: E402
import concourse.tile as tile  # noqa: E402
from concourse import bacc, mybir  # noqa: E402
from concourse.masks import make_identity  # noqa: E402

FP32 = mybir.dt.float32
BF16 = mybir.dt.bfloat16

NCORES = 8
B, T, F, U = 64, 512, 1024, 1024
BC = B // NCORES          # batch per core = 8
K = 16                    # block length
NB = T // K               # 32 blocks
R_LANES = NB * BC         # 256 scan lanes
P = 128                   # partitions
FC = F // P               # 8 f-chunks
UC = U // P               # 8 u-chunks


def build_nc():
    nc = bacc.Bacc("TRN2", target_bir_lowering=False, debug=False,
                   num_devices=NCORES)

    seq = nc.dram_tensor("seq", [BC, T, F], FP32, kind="ExternalInput").ap()
    w_in = nc.dram_tensor("w", [F, U], FP32, kind="ExternalInput").ap()
    r_in = nc.dram_tensor("r", [U, U], FP32, kind="ExternalInput").ap()
    h0_in = nc.dram_tensor("h0", [BC, U], FP32, kind="ExternalInput").ap()
    reps_in = nc.dram_tensor("reps", [1, 1], mybir.dt.int32,
                             kind="ExternalInput").ap()
    out = nc.dram_tensor("out", [BC, T, U], FP32, kind="ExternalOutput").ap()
    # bf16 copy of seq, w-major per jj-chunk: xbf[jj, w, f], w = blk*16+j2*8+b
    # (t = blk*16 + jj*2 + j2)
    xbf = nc.dram_tensor("xbf", [8, 512, F], BF16).ap()
    # squaring-chain transpose bounce: pw[i] holds R^(2^i) natural (bf16)
    pw = nc.dram_tensor("pw", [6, U, U], BF16).ap()
    # output bounce scratch (bf16, transposed [u, j, r])
    osc_d = nc.dram_tensor("out_scratch", [U, K, R_LANES], BF16).ap()

    # DRAM views
    seq5 = seq.rearrange("b (blk jj j2) f -> jj blk j2 b f", blk=NB, jj=8, j2=2)
    xbf_w = xbf.rearrange("jj (blk j2 b) f -> jj blk j2 b f", blk=NB, j2=2)
    osc_v = osc_d.rearrange("(m p) j r -> p m j r", p=P)
    osc_flat = osc_d.rearrange("u j r -> u (j r)")           # [1024, 4096]
    out4 = out.rearrange("b (nb k) u -> nb b k u", k=K)      # [32, 8, 16, 1024]

    with tile.TileContext(nc) as tc:
        def _body(_it=None):
            with (
                tc.tile_pool(name="consts", bufs=1) as consts,
                tc.tile_pool(name="mats", bufs=1) as mats,
                tc.tile_pool(name="xpsb", bufs=1) as xpsb_p,
                tc.tile_pool(name="psA", bufs=4, space="PSUM") as psA,
                tc.tile_pool(name="psT", bufs=2, space="PSUM") as psT,
            ):
                id8 = consts.tile([BC, BC], FP32)
                make_identity(nc, id8)

                # ---- load W, R (cast fp32->bf16 during DMA) ----------------
                # mats pool rotating slots: r, s0 (transposed power), s1, s2.
                # W shares slot s2 (dead after xproj, before s2's first write
                # at R^4 -- the WAR dep orders R^4 after X's last matmul).
                w_sb = mats.tile([P, FC, U], BF16, tag="s2")
                r_sb = mats.tile([P, UC, U], BF16, tag="r")
                nc.gpsimd.dma_start(
                    out=w_sb, in_=w_in.rearrange("(k p) u -> p k u", p=P))
                nc.gpsimd.dma_start(
                    out=r_sb, in_=r_in.rearrange("(k p) u -> p k u", p=P))

                # ---- X input cast: seq fp32 -> xbf bf16 (w-major) ----------
                # split per (jj, j2): DMA AP balancing is limited to 3 dims
                for jj in range(8):
                    for j2 in range(2):
                        nc.gpsimd.dma_start(out=xbf_w[jj, :, j2],
                                            in_=seq5[jj, :, j2])

                # ---- Phase X: xproj -> xpsb (SBUF-resident, bf16) ----------
                xpsb = xpsb_p.tile([P, UC, K, R_LANES], BF16)
                with tc.tile_pool(name="xt", bufs=2) as xt_p:
                    for jj in range(8):
                        xt = xt_p.tile([P, FC, 512], BF16, tag="xt")
                        for c in range(FC):
                            nc.sync.dma_start(
                                out=xt[:, c, :],
                                in_=xbf[jj][:, c * P:(c + 1) * P],
                                transpose=True)
                        for m in range(UC):
                            ps = psA.tile([P, 512], FP32)
                            for k in range(FC):
                                nc.tensor.matmul(
                                    ps, w_sb[:, k, m * P:(m + 1) * P],
                                    xt[:, k, :],
                                    start=(k == 0), stop=(k == FC - 1),
                                )
                            # scatter psum (blk,j2,b) -> xpsb[(j2),(blk,b)]
                            nc.vector.tensor_copy(
                                xpsb[:, m, 2 * jj:2 * jj + 2, :].rearrange(
                                    "p j (blk b) -> p j blk b", blk=NB),
                                ps.rearrange("p (blk j2 b) -> p j2 blk b",
                                             blk=NB, j2=2, b=BC),
                            )

                # ---- Phase S: R^16 via squarings; transposes via DMA -------
                def gemm1024(dst, lhsT_t, rhs_t):
                    for m in range(UC):
                        for n in range(2):
                            ps = psA.tile([P, 512], FP32)
                            for k in range(UC):
                                nc.tensor.matmul(
                                    ps, lhsT_t[:, k, m * P:(m + 1) * P],
                                    rhs_t[:, k, n * 512:(n + 1) * 512],
                                    start=(k == 0), stop=(k == UC - 1),
                                )
                            nc.vector.tensor_copy(
                                dst[:, m, n * 512:(n + 1) * 512], ps)

                def dma_transpose1024(dst, src_sb, slot):
                    # store src (tiled [p,k,u] bf16) -> pw[slot] natural, then
                    # xbar-transpose-read back into dst (tiled [p,k,u] = src.T)
                    nc.scalar.dma_start(
                        out=pw[slot].rearrange("(k p) u -> p k u", p=P),
                        in_=src_sb)
                    for c in range(UC):
                        nc.sync.dma_start(
                            out=dst[:, c, :],
                            in_=pw[slot][:, c * P:(c + 1) * P],
                            transpose=True)

                rt_t = mats.tile([P, UC, U], BF16, tag="s0")
                dma_transpose1024(rt_t, r_sb, 0)
                r2 = mats.tile([P, UC, U], BF16, tag="s1")
                gemm1024(r2, rt_t, r_sb)                  # R^2
                r2t = mats.tile([P, UC, U], BF16, tag="s0")
                dma_transpose1024(r2t, r2, 1)
                r4 = mats.tile([P, UC, U], BF16, tag="s2")
                gemm1024(r4, r2t, r2)                     # R^4
                r4t = mats.tile([P, UC, U], BF16, tag="s0")
                dma_transpose1024(r4t, r4, 2)
                r8 = mats.tile([P, UC, U], BF16, tag="s1")
                gemm1024(r8, r4t, r4)                     # R^8
                r8t = mats.tile([P, UC, U], BF16, tag="s0")
                dma_transpose1024(r8t, r8, 3)
                r16 = mats.tile([P, UC, U], BF16, tag="s2")
                gemm1024(r16, r8t, r8)                    # R^16
                r16t = mats.tile([P, UC, U], BF16, tag="s0")
                dma_transpose1024(r16t, r16, 4)
                r32 = mats.tile([P, UC, U], BF16, tag="s1")
                gemm1024(r32, r16t, r16)                  # R^32
                r32t = mats.tile([P, UC, U], BF16, tag="s0")
                dma_transpose1024(r32t, r32, 5)
                r64 = mats.tile([P, UC, U], BF16, tag="s3")
                gemm1024(r64, r32t, r32)                  # R^64

                # ---- Phase A: zero-init batched scan -----------------------
                with (
                    tc.tile_pool(name="st", bufs=2) as st_p,
                    tc.tile_pool(name="psb", bufs=1) as psb_p,
                ):
                    st_prev = xpsb[:, :, 0, :]
                    for j in range(1, K):
                        st_new = st_p.tile([P, UC, R_LANES], BF16, tag="st")
                        for m in range(UC):
                            ps = psA.tile([P, R_LANES], FP32)
                            for k in range(UC):
                                nc.tensor.matmul(
                                    ps, r_sb[:, k, m * P:(m + 1) * P],
                                    st_prev[:, k, :],
                                    start=(k == 0), stop=(k == UC - 1),
                                )
                            nc.vector.tensor_add(st_new[:, m, :], ps,
                                                 xpsb[:, m, j, :])
                        st_prev = st_new
                    psb = psb_p.tile([P, UC, R_LANES], BF16)
                    nc.scalar.copy(psb, st_prev)

                    # ---- Phase B: two-level boundary scan ------------------
                    # NS=8 supergroups of G=4 blocks; lane nb = s*G + g.
                    # L1: zero-init partials Y_{s,i} batched over 64 lanes
                    # L2: 7-step chain over supers with R^64 -> hall (H_s)
                    # L3: corrections W_i = H @ R16^i, batched -> ci
                    G = 4
                    NS = NB // G
                    SL = NS * BC  # 64 super-lanes
                    ylv = psb.rearrange("p u (ns g b) -> p u ns g b", g=G, b=BC)
                    with (
                        tc.tile_pool(name="yb", bufs=1) as yb_p,
                        tc.tile_pool(name="wb", bufs=2) as wb_p,
                        tc.tile_pool(name="ci", bufs=1) as ci_p,
                        tc.tile_pool(name="mn", bufs=2) as mn_p,
                    ):
                        # ---- L1: Y chain (Y_1 = psb g0; Y_i = psb g(i-1)
                        #                   + Y_{i-1} @ R16) ----
                        y1 = yb_p.tile([P, UC, SL], BF16, tag="y1")
                        nc.vector.tensor_copy(
                            y1.rearrange("p u (ns b) -> p u ns b", b=BC),
                            ylv[:, :, :, 0, :])
                        ys = {1: y1}
                        ycur = y1
                        for i in range(2, G + 1):
                            ynew = yb_p.tile([P, UC, SL], BF16, tag=f"y{i}")
                            for m in range(UC):
                                ps = psA.tile([P, SL], FP32)
                                for k in range(UC):
                                    nc.tensor.matmul(
                                        ps, r16[:, k, m * P:(m + 1) * P],
                                        ycur[:, k, :],
                                        start=(k == 0), stop=(k == UC - 1),
                                    )
                                nc.vector.tensor_add(
                                    ynew[:, m, :].rearrange(
                                        "p (ns b) -> p ns b", b=BC),
                                    ps.rearrange("p (ns b) -> p ns b", b=BC),
                                    ylv[:, m, :, i - 1, :])
                            ys[i] = ynew
                            ycur = ynew

                        # ---- L2: H chain over supers (R^64 jumps) ----
                        hall = yb_p.tile([P, UC, SL], BF16, tag="hall")
                        h0sb = mn_p.tile([BC, U], FP32, tag="h0")
                        nc.sync.dma_start(out=h0sb, in_=h0_in)
                        for c in range(UC):
                            pt = psT.tile([P, BC], FP32)
                            nc.tensor.transpose(pt, h0sb[:, c * P:(c + 1) * P], id8)
                            nc.scalar.copy(hall[:, c, 0:BC], pt)
                        y4 = ys[G]
                        for s in range(NS - 1):
                            mn = mn_p.tile([BC, U], FP32, tag="mn")
                            for n in range(2):
                                ps = psA.tile([BC, 512], FP32)
                                for k in range(UC):
                                    nc.tensor.matmul(
                                        ps, hall[:, k, s * BC:(s + 1) * BC],
                                        r64[:, k, n * 512:(n + 1) * 512],
                                        start=(k == 0), stop=(k == UC - 1),
                                    )
                                nc.scalar.copy(mn[:, n * 512:(n + 1) * 512], ps)
                            for c in range(UC):
                                pt = psT.tile([P, BC], FP32)
                                nc.tensor.transpose(pt, mn[:, c * P:(c + 1) * P], id8)
                                nc.vector.tensor_add(
                                    hall[:, c, (s + 1) * BC:(s + 2) * BC], pt,
                                    y4[:, c, s * BC:(s + 1) * BC])

                        # ---- L3: corrections + assemble ci ----
                        ci = ci_p.tile([P, UC, R_LANES], BF16)
                        civ = ci.rearrange("p u (ns g b) -> p u ns g b",
                                           g=G, b=BC)
                        nc.vector.tensor_copy(
                            civ[:, :, :, 0, :],
                            hall.rearrange("p u (ns b) -> p u ns b", b=BC))
                        wcur = hall
                        for i in range(1, G):
                            wnew = wb_p.tile([P, UC, SL], BF16, tag="wrot")
                            for m in range(UC):
                                ps = psA.tile([P, SL], FP32)
                                for k in range(UC):
                                    nc.tensor.matmul(
                                        ps, r16[:, k, m * P:(m + 1) * P],
                                        wcur[:, k, :],
                                        start=(k == 0), stop=(k == UC - 1),
                                    )
                                nc.vector.tensor_copy(wnew[:, m, :], ps)
                                if i == 1:
                                    nc.vector.tensor_add(
                                        civ[:, m, :, 1, :],
                                        ps.rearrange("p (ns b) -> p ns b", b=BC),
                                        ylv[:, m, :, 0, :])
                                else:
                                    nc.vector.tensor_add(
                                        civ[:, m, :, i, :],
                                        ps.rearrange("p (ns b) -> p ns b", b=BC),
                                        ys[i].rearrange(
                                            "p u (ns b) -> p u ns b",
                                            b=BC)[:, m, :, :])
                            wcur = wnew

                        # ---- Phase C: corrected scan + outputs -------------
                        # per j: st -> osc (DRAM bf16) -> DMA-transpose into
                        # out_acc [lane, jg, u]; after each 8-j group, one big
                        # cast-store per lane-half (64KB-contiguous runs).
                        with tc.tile_pool(name="oacc", bufs=1) as oacc_p:
                            st_prev = ci
                            for g in range(4):
                                oacc = oacc_p.tile([P, 2, K // 4, U], BF16,
                                                   tag="oacc")
                                for jg in range(K // 4):
                                    j = g * (K // 4) + jg
                                    st_new = st_p.tile([P, UC, R_LANES], BF16,
                                                       tag="st")
                                    for m in range(UC):
                                        ps = psA.tile([P, R_LANES], FP32)
                                        for k in range(UC):
                                            nc.tensor.matmul(
                                                ps, r_sb[:, k, m * P:(m + 1) * P],
                                                st_prev[:, k, :],
                                                start=(k == 0), stop=(k == UC - 1),
                                            )
                                        nc.vector.tensor_add(
                                            st_new[:, m, :], ps, xpsb[:, m, j, :])
                                    st_prev = st_new
                                    nc.scalar.dma_start(
                                        out=osc_v[:, :, j, :], in_=st_new)
                                    for h in range(2):
                                        w = 2 * j + h
                                        nc.sync.dma_start(
                                            out=oacc[:, h, jg, :],
                                            in_=osc_flat[:, w * P:(w + 1) * P],
                                            transpose=True)
                                for h in range(2):
                                    nc.gpsimd.dma_start(
                                        out=out4[h * 16:(h + 1) * 16, :,
                                                 g * (K // 4):(g + 1) * (K // 4), :],
                                        in_=oacc[:, h, :, :],
                                    )

        with tc.tile_pool(name="repsp", bufs=1) as reps_p:
            rtile = reps_p.tile([1, 1], mybir.dt.int32)
            nc.sync.dma_start(out=rtile, in_=reps_in)
            reps_val = nc.values_load(rtile[0:1, 0:1])
            with tc.For_i(0, reps_val, 1) as _it:
                _body(_it)

    nc.compile()
    return nc


_NC_CACHE = {}


def _get_nc(reps=1):
    if "nc" not in _NC_CACHE:
        _NC_CACHE["nc"] = build_nc()
    return _NC_CACHE["nc"]


def _make_in_maps(sequences, kernel, recurrent_kernel, h0, reps=1):
    in_maps = []
    for c in range(NCORES):
        sl = slice(c * BC, (c + 1) * BC)
        in_maps.append({
            "seq": sequences[sl],
            "w": kernel,
            "r": recurrent_kernel,
            "h0": h0[sl],
            "reps": np.array([[reps]], dtype=np.int32),
        })
    return in_maps


def bench(inputs, reps):
    from concourse.bass_utils import run_bass_kernel_spmd
    nc = _get_nc()
    in_maps = _make_in_maps(
        np.ascontiguousarray(inputs["sequences"], dtype=np.float32),
        np.ascontiguousarray(inputs["kernel"], dtype=np.float32),
        np.ascontiguousarray(inputs["recurrent_kernel"], dtype=np.float32),
        np.ascontiguousarray(inputs["h0"], dtype=np.float32), reps)
    return run_bass_kernel_spmd(nc, in_maps, core_ids=list(range(NCORES)))


def kernel(sequences, kernel, recurrent_kernel, h0):
    from concourse.bass_utils import run_bass_kernel_spmd
    nc = _get_nc()
    sequences = np.ascontiguousarray(sequences, dtype=np.float32)
    kernel = np.ascontiguousarray(kernel, dtype=np.float32)
    recurrent_kernel = np.ascontiguousarray(recurrent_kernel, dtype=np.float32)
    h0 = np.ascontiguousarray(h0, dtype=np.float32)
    in_maps = _make_in_maps(sequences, kernel, recurrent_kernel, h0)
    res = run_bass_kernel_spmd(nc, in_maps, core_ids=list(range(NCORES)))
    return np.concatenate([res.results[c]["out"] for c in range(NCORES)], axis=0)


# ---------------------------------------------------------------- dev tools
def _numpy_model(seqs, W, R, h0):
    xp = seqs.reshape(-1, F) @ W
    xp = xp.reshape(seqs.shape[0], T, U)
    h = h0.copy()
    outs = np.zeros((seqs.shape[0], T, U), np.float32)
    for t in range(T):
        h = xp[:, t] + h @ R
        outs[:, t] = h
    return outs


# revision 5
# speedup vs baseline: 1.1893x; 1.1893x over previous
"""Trainium2 Bass kernel for nn_RNN: h_t = x_t @ W + h_{t-1} @ R (linear RNN). v2.

Full shapes: sequences [64, 512, 1024], kernel [1024, 1024],
recurrent_kernel [1024, 1024], h0 [64, 1024] -> out [64, 512, 1024].
Sharding: data-parallel over batch across 8 cores (8 sequences/core).

Per-core blocked scan, K=16 block length, NB=32 blocks, lane r = blk*8+b.
v2 changes vs v1:
  - xproj kept SBUF-resident (bf16, 64KB/partition); no DRAM xp bounce.
  - X transposes via DMA xbar (seq cast to bf16 time-major in DRAM by a
    SWDGE cast DMA, then hardware DMA-transpose loads), not PE.
  - squaring-chain transposes via DRAM bf16 bounce + DMA-transpose, not PE.
  - output path: osc bf16 bounce (ACT queue) -> DMA-transpose (SP queue)
    -> SWDGE cast bf16->fp32 store (Pool queue); no DVE copy.
"""
import sys
import numpy as np

sys.path.insert(0, "/opt/trn_rl_repo")

try:  # persistent jit cache: repeated kernel() invocations skip recompile
    import jax
    import os as _os
    _cache = _os.environ.get("JAX_COMPILATION_CACHE_DIR", "/tmp/jaxcache_rnn")
    _os.makedirs(_cache, exist_ok=True)
    jax.config.update("jax_compilation_cache_dir", _cache)
except Exception:
    pass

import concourse.bass as bass  # noqa: E402
import concourse.tile as tile  # noqa: E402
from concourse import bacc, mybir  # noqa: E402
from concourse.masks import make_identity  # noqa: E402

FP32 = mybir.dt.float32
BF16 = mybir.dt.bfloat16

NCORES = 8
B, T, F, U = 64, 512, 1024, 1024
BC = B // NCORES          # batch per core = 8
K = 16                    # block length
NB = T // K               # 32 blocks
R_LANES = NB * BC         # 256 scan lanes
P = 128                   # partitions
FC = F // P               # 8 f-chunks
UC = U // P               # 8 u-chunks


def build_nc():
    nc = bacc.Bacc("TRN2", target_bir_lowering=False, debug=False,
                   num_devices=NCORES)

    seq = nc.dram_tensor("seq", [BC, T, F], FP32, kind="ExternalInput").ap()
    w_in = nc.dram_tensor("w", [F, U], FP32, kind="ExternalInput").ap()
    r_in = nc.dram_tensor("r", [U, U], FP32, kind="ExternalInput").ap()
    h0_in = nc.dram_tensor("h0", [BC, U], FP32, kind="ExternalInput").ap()
    reps_in = nc.dram_tensor("reps", [1, 1], mybir.dt.int32,
                             kind="ExternalInput").ap()
    out = nc.dram_tensor("out", [BC, T, U], FP32, kind="ExternalOutput").ap()
    # bf16 copy of seq, w-major per jj-chunk: xbf[jj, w, f], w = blk*16+j2*8+b
    # (t = blk*16 + jj*2 + j2)
    xbf = nc.dram_tensor("xbf", [8, 512, F], BF16).ap()
    # squaring-chain transpose bounce: pw[i] holds R^(2^i) natural (bf16)
    pw = nc.dram_tensor("pw", [4, U, U], BF16).ap()
    # output bounce scratch (bf16, transposed [u, j, r])
    osc_d = nc.dram_tensor("out_scratch", [U, K, R_LANES], BF16).ap()

    # DRAM views
    seq5 = seq.rearrange("b (blk jj j2) f -> jj blk j2 b f", blk=NB, jj=8, j2=2)
    xbf_w = xbf.rearrange("jj (blk j2 b) f -> jj blk j2 b f", blk=NB, j2=2)
    osc_v = osc_d.rearrange("(m p) j r -> p m j r", p=P)
    osc_flat = osc_d.rearrange("u j r -> u (j r)")           # [1024, 4096]
    out4 = out.rearrange("b (nb k) u -> nb b k u", k=K)      # [32, 8, 16, 1024]

    with tile.TileContext(nc) as tc:
        def _body(_it=None):
            with (
                tc.tile_pool(name="consts", bufs=1) as consts,
                tc.tile_pool(name="mats", bufs=1) as mats,
                tc.tile_pool(name="xpsb", bufs=1) as xpsb_p,
                tc.tile_pool(name="psA", bufs=4, space="PSUM") as psA,
                tc.tile_pool(name="psT", bufs=2, space="PSUM") as psT,
            ):
                id8 = consts.tile([BC, BC], FP32)
                make_identity(nc, id8)

                # ---- load W, R (cast fp32->bf16 during DMA) ----------------
                # mats pool rotating slots: r, s0 (transposed power), s1, s2.
                # W shares slot s2 (dead after xproj, before s2's first write
                # at R^4 -- the WAR dep orders R^4 after X's last matmul).
                w_sb = mats.tile([P, FC, U], BF16, tag="s2")
                r_sb = mats.tile([P, UC, U], BF16, tag="r")
                nc.gpsimd.dma_start(
                    out=w_sb, in_=w_in.rearrange("(k p) u -> p k u", p=P))
                nc.gpsimd.dma_start(
                    out=r_sb, in_=r_in.rearrange("(k p) u -> p k u", p=P))

                # ---- X input cast: seq fp32 -> xbf bf16 (w-major) ----------
                # split per (jj, j2): DMA AP balancing is limited to 3 dims
                for jj in range(8):
                    for j2 in range(2):
                        nc.gpsimd.dma_start(out=xbf_w[jj, :, j2],
                                            in_=seq5[jj, :, j2])

                # ---- Phase X: xproj -> xpsb (SBUF-resident, bf16) ----------
                xpsb = xpsb_p.tile([P, UC, K, R_LANES], BF16)
                with tc.tile_pool(name="xt", bufs=2) as xt_p:
                    for jj in range(8):
                        xt = xt_p.tile([P, FC, 512], BF16, tag="xt")
                        for c in range(FC):
                            nc.sync.dma_start(
                                out=xt[:, c, :],
                                in_=xbf[jj][:, c * P:(c + 1) * P],
                                transpose=True)
                        for m in range(UC):
                            ps = psA.tile([P, 512], FP32)
                            for k in range(FC):
                                nc.tensor.matmul(
                                    ps, w_sb[:, k, m * P:(m + 1) * P],
                                    xt[:, k, :],
                                    start=(k == 0), stop=(k == FC - 1),
                                )
                            # scatter psum (blk,j2,b) -> xpsb[(j2),(blk,b)]
                            nc.vector.tensor_copy(
                                xpsb[:, m, 2 * jj:2 * jj + 2, :].rearrange(
                                    "p j (blk b) -> p j blk b", blk=NB),
                                ps.rearrange("p (blk j2 b) -> p j2 blk b",
                                             blk=NB, j2=2, b=BC),
                            )

                # ---- Phase S: R^16 via squarings; transposes via DMA -------
                def gemm1024(dst, lhsT_t, rhs_t):
                    for m in range(UC):
                        for n in range(2):
                            ps = psA.tile([P, 512], FP32)
                            for k in range(UC):
                                nc.tensor.matmul(
                                    ps, lhsT_t[:, k, m * P:(m + 1) * P],
                                    rhs_t[:, k, n * 512:(n + 1) * 512],
                                    start=(k == 0), stop=(k == UC - 1),
                                )
                            nc.vector.tensor_copy(
                                dst[:, m, n * 512:(n + 1) * 512], ps)

                def dma_transpose1024(dst, src_sb, slot):
                    # store src (tiled [p,k,u] bf16) -> pw[slot] natural, then
                    # xbar-transpose-read back into dst (tiled [p,k,u] = src.T)
                    nc.scalar.dma_start(
                        out=pw[slot].rearrange("(k p) u -> p k u", p=P),
                        in_=src_sb)
                    for c in range(UC):
                        nc.sync.dma_start(
                            out=dst[:, c, :],
                            in_=pw[slot][:, c * P:(c + 1) * P],
                            transpose=True)

                rt_t = mats.tile([P, UC, U], BF16, tag="s0")
                dma_transpose1024(rt_t, r_sb, 0)
                r2 = mats.tile([P, UC, U], BF16, tag="s1")
                gemm1024(r2, rt_t, r_sb)                  # R^2
                r2t = mats.tile([P, UC, U], BF16, tag="s0")
                dma_transpose1024(r2t, r2, 1)
                r4 = mats.tile([P, UC, U], BF16, tag="s2")
                gemm1024(r4, r2t, r2)                     # R^4
                r4t = mats.tile([P, UC, U], BF16, tag="s0")
                dma_transpose1024(r4t, r4, 2)
                r8 = mats.tile([P, UC, U], BF16, tag="s1")
                gemm1024(r8, r4t, r4)                     # R^8
                r8t = mats.tile([P, UC, U], BF16, tag="s0")
                dma_transpose1024(r8t, r8, 3)
                r16 = mats.tile([P, UC, U], BF16, tag="s2")
                gemm1024(r16, r8t, r8)                    # R^16

                # ---- Phase A: zero-init batched scan -----------------------
                with (
                    tc.tile_pool(name="st", bufs=2) as st_p,
                    tc.tile_pool(name="psb", bufs=1) as psb_p,
                ):
                    st_prev = xpsb[:, :, 0, :]
                    psb = psb_p.tile([P, UC, R_LANES], FP32)
                    for j in range(1, K):
                        # last step writes psb directly (psb IS st at j=15);
                        # m-pairs share one PSUM bank -> one DVE add per pair
                        st_new = (psb if j == K - 1 else
                                  st_p.tile([P, UC, R_LANES], BF16, tag="st"))
                        for m2 in range(UC // 2):
                            m = 2 * m2
                            ps = psA.tile([P, 2 * R_LANES], FP32)
                            for mm in range(2):
                                for k in range(UC):
                                    nc.tensor.matmul(
                                        ps[:, mm * R_LANES:(mm + 1) * R_LANES],
                                        r_sb[:, k, (m + mm) * P:(m + mm + 1) * P],
                                        st_prev[:, k, :],
                                        start=(k == 0), stop=(k == UC - 1),
                                    )
                            nc.vector.tensor_add(
                                st_new[:, m:m + 2, :],
                                ps.rearrange("p (mm r) -> p mm r", mm=2),
                                xpsb[:, m:m + 2, j, :])
                        st_prev = st_new

                    # ---- Phase B: boundary scan (32 blocks, thin) ----------
                    with (
                        tc.tile_pool(name="hbt", bufs=2) as hbt_p,
                        tc.tile_pool(name="ci", bufs=1) as ci_p,
                        tc.tile_pool(name="mn", bufs=2) as mn_p,
                    ):
                        ci = ci_p.tile([P, UC, R_LANES], BF16)
                        h0sb = mn_p.tile([BC, U], FP32, tag="h0")
                        nc.sync.dma_start(out=h0sb, in_=h0_in)
                        hbt = hbt_p.tile([P, UC, BC], BF16, tag="hbt")
                        for c in range(UC):
                            pt = psT.tile([P, BC], FP32)
                            nc.tensor.transpose(pt, h0sb[:, c * P:(c + 1) * P], id8)
                            nc.scalar.copy(hbt[:, c, :], pt)
                            nc.scalar.copy(ci[:, c, 0:BC], pt)
                        for b in range(NB):
                            mn = mn_p.tile([BC, U], FP32, tag="mn")
                            for n in range(2):
                                ps = psA.tile([BC, 512], FP32)
                                for k in range(UC):
                                    nc.tensor.matmul(
                                        ps, hbt[:, k, :],
                                        r16[:, k, n * 512:(n + 1) * 512],
                                        start=(k == 0), stop=(k == UC - 1),
                                    )
                                nc.scalar.copy(mn[:, n * 512:(n + 1) * 512], ps)
                            hbt_n = hbt_p.tile([P, UC, BC], BF16, tag="hbt")
                            for c in range(UC):
                                pt = psT.tile([P, BC], FP32)
                                nc.tensor.transpose(pt, mn[:, c * P:(c + 1) * P], id8)
                                nc.vector.tensor_add(
                                    hbt_n[:, c, :], pt, psb[:, c, b * BC:(b + 1) * BC])
                                if b < NB - 1:
                                    nc.scalar.copy(
                                        ci[:, c, (b + 1) * BC:(b + 2) * BC],
                                        hbt_n[:, c, :])
                            hbt = hbt_n

                        # ---- Phase C: corrected scan + outputs -------------
                        # per j: st -> osc (DRAM bf16) -> DMA-transpose into
                        # out_acc [lane, jg, u]; after each 8-j group, one big
                        # cast-store per lane-half (64KB-contiguous runs).
                        with tc.tile_pool(name="oacc", bufs=1) as oacc_p:
                            st_prev = ci
                            for g in range(2):
                                oacc = oacc_p.tile([P, 2, K // 2, U], BF16,
                                                   tag="oacc")
                                for jg in range(K // 2):
                                    j = g * (K // 2) + jg
                                    st_new = st_p.tile([P, UC, R_LANES], BF16,
                                                       tag="st")
                                    for m2 in range(UC // 2):
                                        m = 2 * m2
                                        ps = psA.tile([P, 2 * R_LANES], FP32)
                                        for mm in range(2):
                                            for k in range(UC):
                                                nc.tensor.matmul(
                                                    ps[:, mm * R_LANES:
                                                       (mm + 1) * R_LANES],
                                                    r_sb[:, k, (m + mm) * P:
                                                         (m + mm + 1) * P],
                                                    st_prev[:, k, :],
                                                    start=(k == 0),
                                                    stop=(k == UC - 1),
                                                )
                                        nc.vector.tensor_add(
                                            st_new[:, m:m + 2, :],
                                            ps.rearrange("p (mm r) -> p mm r",
                                                         mm=2),
                                            xpsb[:, m:m + 2, j, :])
                                    st_prev = st_new
                                    nc.scalar.dma_start(
                                        out=osc_v[:, :, j, :], in_=st_new)
                                    for h in range(2):
                                        w = 2 * j + h
                                        nc.sync.dma_start(
                                            out=oacc[:, h, jg, :],
                                            in_=osc_flat[:, w * P:(w + 1) * P],
                                            transpose=True)
                                for h in range(2):
                                    nc.gpsimd.dma_start(
                                        out=out4[h * 16:(h + 1) * 16, :,
                                                 g * (K // 2):(g + 1) * (K // 2), :],
                                        in_=oacc[:, h, :, :],
                                    )

        with tc.tile_pool(name="repsp", bufs=1) as reps_p:
            rtile = reps_p.tile([1, 1], mybir.dt.int32)
            nc.sync.dma_start(out=rtile, in_=reps_in)
            reps_val = nc.values_load(rtile[0:1, 0:1])
            with tc.For_i(0, reps_val, 1) as _it:
                _body(_it)

    nc.compile()
    return nc


_NC_CACHE = {}


def _get_nc(reps=1):
    if "nc" not in _NC_CACHE:
        _NC_CACHE["nc"] = build_nc()
    return _NC_CACHE["nc"]


def _make_in_maps(sequences, kernel, recurrent_kernel, h0, reps=1):
    in_maps = []
    for c in range(NCORES):
        sl = slice(c * BC, (c + 1) * BC)
        in_maps.append({
            "seq": sequences[sl],
            "w": kernel,
            "r": recurrent_kernel,
            "h0": h0[sl],
            "reps": np.array([[reps]], dtype=np.int32),
        })
    return in_maps


def bench(inputs, reps):
    from concourse.bass_utils import run_bass_kernel_spmd
    nc = _get_nc()
    in_maps = _make_in_maps(
        np.ascontiguousarray(inputs["sequences"], dtype=np.float32),
        np.ascontiguousarray(inputs["kernel"], dtype=np.float32),
        np.ascontiguousarray(inputs["recurrent_kernel"], dtype=np.float32),
        np.ascontiguousarray(inputs["h0"], dtype=np.float32), reps)
    return run_bass_kernel_spmd(nc, in_maps, core_ids=list(range(NCORES)))


def kernel(sequences, kernel, recurrent_kernel, h0):
    from concourse.bass_utils import run_bass_kernel_spmd
    nc = _get_nc()
    sequences = np.ascontiguousarray(sequences, dtype=np.float32)
    kernel = np.ascontiguousarray(kernel, dtype=np.float32)
    recurrent_kernel = np.ascontiguousarray(recurrent_kernel, dtype=np.float32)
    h0 = np.ascontiguousarray(h0, dtype=np.float32)
    in_maps = _make_in_maps(sequences, kernel, recurrent_kernel, h0)
    res = run_bass_kernel_spmd(nc, in_maps, core_ids=list(range(NCORES)))
    return np.concatenate([res.results[c]["out"] for c in range(NCORES)], axis=0)


# ---------------------------------------------------------------- dev tools
def _numpy_model(seqs, W, R, h0):
    xp = seqs.reshape(-1, F) @ W
    xp = xp.reshape(seqs.shape[0], T, U)
    h = h0.copy()
    outs = np.zeros((seqs.shape[0], T, U), np.float32)
    for t in range(T):
        h = xp[:, t] + h @ R
        outs[:, t] = h
    return outs


# revision 6
# speedup vs baseline: 1.5329x; 1.2889x over previous
"""Trainium2 Bass kernel for nn_RNN: h_t = x_t @ W + h_{t-1} @ R (linear RNN). v2.

Full shapes: sequences [64, 512, 1024], kernel [1024, 1024],
recurrent_kernel [1024, 1024], h0 [64, 1024] -> out [64, 512, 1024].
Sharding: data-parallel over batch across 8 cores (8 sequences/core).

Per-core blocked scan, K=16 block length, NB=32 blocks, lane r = blk*8+b.
v2 changes vs v1:
  - xproj kept SBUF-resident (bf16, 64KB/partition); no DRAM xp bounce.
  - X transposes via DMA xbar (seq cast to bf16 time-major in DRAM by a
    SWDGE cast DMA, then hardware DMA-transpose loads), not PE.
  - squaring-chain transposes via DRAM bf16 bounce + DMA-transpose, not PE.
  - output path: osc bf16 bounce (ACT queue) -> DMA-transpose (SP queue)
    -> SWDGE cast bf16->fp32 store (Pool queue); no DVE copy.
"""
import sys
import numpy as np

sys.path.insert(0, "/opt/trn_rl_repo")

try:  # persistent jit cache: repeated kernel() invocations skip recompile
    import jax
    import os as _os
    _cache = _os.environ.get("JAX_COMPILATION_CACHE_DIR", "/tmp/jaxcache_rnn")
    _os.makedirs(_cache, exist_ok=True)
    jax.config.update("jax_compilation_cache_dir", _cache)
except Exception:
    pass

import concourse.bass as bass  # noqa: E402
import concourse.tile as tile  # noqa: E402
from concourse import bacc, mybir  # noqa: E402
from concourse.masks import make_identity  # noqa: E402

FP32 = mybir.dt.float32
BF16 = mybir.dt.bfloat16

NCORES = 8
B, T, F, U = 64, 512, 1024, 1024
BC = B // NCORES          # batch per core = 8
K = 16                    # block length
NB = T // K               # 32 blocks
R_LANES = NB * BC         # 256 scan lanes
P = 128                   # partitions
FC = F // P               # 8 f-chunks
UC = U // P               # 8 u-chunks


def build_nc():
    nc = bacc.Bacc("TRN2", target_bir_lowering=False, debug=False,
                   num_devices=NCORES)

    seq = nc.dram_tensor("seq", [BC, T, F], FP32, kind="ExternalInput").ap()
    w_in = nc.dram_tensor("w", [F, U], FP32, kind="ExternalInput").ap()
    r_in = nc.dram_tensor("r", [U, U], FP32, kind="ExternalInput").ap()
    h0_in = nc.dram_tensor("h0", [BC, U], FP32, kind="ExternalInput").ap()
    reps_in = nc.dram_tensor("reps", [1, 1], mybir.dt.int32,
                             kind="ExternalInput").ap()
    out = nc.dram_tensor("out", [BC, T, U], FP32, kind="ExternalOutput").ap()
    # bf16 copy of seq, w-major per jj-chunk: xbf[jj, w, f], w = blk*16+j2*8+b
    # (t = blk*16 + jj*2 + j2)
    xbf = nc.dram_tensor("xbf", [8, 512, F], BF16).ap()
    # squaring-chain transpose bounce: pw[i] holds R^(2^i) natural (bf16)
    pw = nc.dram_tensor("pw", [4, U, U], BF16).ap()
    # output bounce scratch (bf16, transposed [u, j, r])
    osc_d = nc.dram_tensor("out_scratch", [U, K, R_LANES], BF16).ap()

    # DRAM views
    seq5 = seq.rearrange("b (blk jj j2) f -> jj blk j2 b f", blk=NB, jj=8, j2=2)
    xbf_w = xbf.rearrange("jj (blk j2 b) f -> jj blk j2 b f", blk=NB, j2=2)
    osc_v = osc_d.rearrange("(m p) j r -> p m j r", p=P)
    osc_flat = osc_d.rearrange("u j r -> u (j r)")           # [1024, 4096]
    out4 = out.rearrange("b (nb k) u -> nb b k u", k=K)      # [32, 8, 16, 1024]

    with tile.TileContext(nc) as tc:
        def _body(_it=None):
            with (
                tc.tile_pool(name="consts", bufs=1) as consts,
                tc.tile_pool(name="mats", bufs=1) as mats,
                tc.tile_pool(name="xpsb", bufs=1) as xpsb_p,
                tc.tile_pool(name="psA", bufs=4, space="PSUM") as psA,
                tc.tile_pool(name="psT", bufs=2, space="PSUM") as psT,
            ):
                id8 = consts.tile([BC, BC], FP32)
                make_identity(nc, id8)

                # ---- load W, R (cast fp32->bf16 during DMA) ----------------
                # mats pool rotating slots: r, s0 (transposed power), s1, s2.
                # W shares slot s2 (dead after xproj, before s2's first write
                # at R^4 -- the WAR dep orders R^4 after X's last matmul).
                w_sb = mats.tile([P, FC, U], BF16, tag="s2")
                r_sb = mats.tile([P, UC, U], BF16, tag="r")
                nc.gpsimd.dma_start(
                    out=w_sb, in_=w_in.rearrange("(k p) u -> p k u", p=P))
                nc.gpsimd.dma_start(
                    out=r_sb, in_=r_in.rearrange("(k p) u -> p k u", p=P))

                # ---- X input cast: seq fp32 -> xbf bf16 (w-major) ----------
                # split per (jj, j2): DMA AP balancing is limited to 3 dims
                for jj in range(8):
                    for j2 in range(2):
                        nc.gpsimd.dma_start(out=xbf_w[jj, :, j2],
                                            in_=seq5[jj, :, j2])

                # ---- Phase X: xproj -> xpsb (SBUF-resident, bf16) ----------
                xpsb = xpsb_p.tile([P, UC, K, R_LANES], BF16)
                with tc.tile_pool(name="xt", bufs=2) as xt_p:
                    for jj in range(8):
                        xt = xt_p.tile([P, FC, 512], BF16, tag="xt")
                        for c in range(FC):
                            nc.sync.dma_start(
                                out=xt[:, c, :],
                                in_=xbf[jj][:, c * P:(c + 1) * P],
                                transpose=True)
                        for m in range(UC):
                            ps = psA.tile([P, 512], FP32)
                            for k in range(FC):
                                nc.tensor.matmul(
                                    ps, w_sb[:, k, m * P:(m + 1) * P],
                                    xt[:, k, :],
                                    start=(k == 0), stop=(k == FC - 1),
                                )
                            # scatter psum (blk,j2,b) -> xpsb[(j2),(blk,b)]
                            nc.vector.tensor_copy(
                                xpsb[:, m, 2 * jj:2 * jj + 2, :].rearrange(
                                    "p j (blk b) -> p j blk b", blk=NB),
                                ps.rearrange("p (blk j2 b) -> p j2 blk b",
                                             blk=NB, j2=2, b=BC),
                            )

                # ---- Phase S: R^16 via squarings; transposes via DMA -------
                def gemm1024(dst, lhsT_t, rhs_t):
                    for m in range(UC):
                        for n in range(2):
                            ps = psA.tile([P, 512], FP32)
                            for k in range(UC):
                                nc.tensor.matmul(
                                    ps, lhsT_t[:, k, m * P:(m + 1) * P],
                                    rhs_t[:, k, n * 512:(n + 1) * 512],
                                    start=(k == 0), stop=(k == UC - 1),
                                )
                            nc.vector.tensor_copy(
                                dst[:, m, n * 512:(n + 1) * 512], ps)

                def dma_transpose1024(dst, src_sb, slot):
                    # store src (tiled [p,k,u] bf16) -> pw[slot] natural, then
                    # xbar-transpose-read back into dst (tiled [p,k,u] = src.T)
                    nc.scalar.dma_start(
                        out=pw[slot].rearrange("(k p) u -> p k u", p=P),
                        in_=src_sb)
                    for c in range(UC):
                        nc.sync.dma_start(
                            out=dst[:, c, :],
                            in_=pw[slot][:, c * P:(c + 1) * P],
                            transpose=True)

                rt_t = mats.tile([P, UC, U], BF16, tag="s0")
                dma_transpose1024(rt_t, r_sb, 0)
                r2 = mats.tile([P, UC, U], BF16, tag="s1")
                gemm1024(r2, rt_t, r_sb)                  # R^2
                r2t = mats.tile([P, UC, U], BF16, tag="s0")
                dma_transpose1024(r2t, r2, 1)
                r4 = mats.tile([P, UC, U], BF16, tag="s2")
                gemm1024(r4, r2t, r2)                     # R^4
                r4t = mats.tile([P, UC, U], BF16, tag="s0")
                dma_transpose1024(r4t, r4, 2)
                r8 = mats.tile([P, UC, U], BF16, tag="s1")
                gemm1024(r8, r4t, r4)                     # R^8
                r8t = mats.tile([P, UC, U], BF16, tag="s0")
                dma_transpose1024(r8t, r8, 3)
                r16 = mats.tile([P, UC, U], BF16, tag="s2")
                gemm1024(r16, r8t, r8)                    # R^16

                # ---- Phase A: zero-init batched scan -----------------------
                with (
                    tc.tile_pool(name="st", bufs=2) as st_p,
                    tc.tile_pool(name="psb", bufs=1) as psb_p,
                ):
                    st_prev = xpsb[:, :, 0, :]
                    for j in range(1, K):
                        st_new = st_p.tile([P, UC, R_LANES], BF16, tag="st")
                        for m in range(UC):
                            ps = psA.tile([P, R_LANES], FP32)
                            for k in range(UC):
                                nc.tensor.matmul(
                                    ps, r_sb[:, k, m * P:(m + 1) * P],
                                    st_prev[:, k, :],
                                    start=(k == 0), stop=(k == UC - 1),
                                )
                            nc.vector.tensor_add(st_new[:, m, :], ps,
                                                 xpsb[:, m, j, :])
                        st_prev = st_new
                    psb = psb_p.tile([P, UC, R_LANES], FP32)
                    nc.scalar.copy(psb, st_prev)

                    # ---- Phase B: boundary scan (32 blocks, thin) ----------
                    with (
                        tc.tile_pool(name="hbt", bufs=2) as hbt_p,
                        tc.tile_pool(name="ci", bufs=1) as ci_p,
                        tc.tile_pool(name="mn", bufs=2) as mn_p,
                    ):
                        ci = ci_p.tile([P, UC, R_LANES], BF16)
                        h0sb = mn_p.tile([BC, U], FP32, tag="h0")
                        nc.sync.dma_start(out=h0sb, in_=h0_in)
                        hbt = hbt_p.tile([P, UC, BC], BF16, tag="hbt")
                        for c in range(UC):
                            pt = psT.tile([P, BC], FP32)
                            nc.tensor.transpose(pt, h0sb[:, c * P:(c + 1) * P], id8)
                            nc.scalar.copy(hbt[:, c, :], pt)
                            nc.scalar.copy(ci[:, c, 0:BC], pt)
                        for b in range(NB):
                            mn = mn_p.tile([BC, U], FP32, tag="mn")
                            for n in range(2):
                                ps = psA.tile([BC, 512], FP32)
                                for k in range(UC):
                                    nc.tensor.matmul(
                                        ps, hbt[:, k, :],
                                        r16[:, k, n * 512:(n + 1) * 512],
                                        start=(k == 0), stop=(k == UC - 1),
                                    )
                                nc.scalar.copy(mn[:, n * 512:(n + 1) * 512], ps)
                            hbt_n = hbt_p.tile([P, UC, BC], BF16, tag="hbt")
                            for c in range(UC):
                                pt = psT.tile([P, BC], FP32)
                                nc.tensor.transpose(pt, mn[:, c * P:(c + 1) * P], id8)
                                nc.vector.tensor_add(
                                    hbt_n[:, c, :], pt, psb[:, c, b * BC:(b + 1) * BC])
                                if b < NB - 1:
                                    nc.scalar.copy(
                                        ci[:, c, (b + 1) * BC:(b + 2) * BC],
                                        hbt_n[:, c, :])
                            hbt = hbt_n

                        # ---- Phase C: corrected scan + outputs -------------
                        # per j: st -> osc (DRAM bf16) -> DMA-transpose into
                        # out_acc [lane, jg, u]; after each 8-j group, one big
                        # cast-store per lane-half (64KB-contiguous runs).
                        with tc.tile_pool(name="oacc", bufs=1) as oacc_p:
                            st_prev = ci
                            for g in range(2):
                                oacc = oacc_p.tile([P, 2, K // 2, U], BF16,
                                                   tag="oacc")
                                for jg in range(K // 2):
                                    j = g * (K // 2) + jg
                                    st_new = st_p.tile([P, UC, R_LANES], BF16,
                                                       tag="st")
                                    for m in range(UC):
                                        ps = psA.tile([P, R_LANES], FP32)
                                        for k in range(UC):
                                            nc.tensor.matmul(
                                                ps, r_sb[:, k, m * P:(m + 1) * P],
                                                st_prev[:, k, :],
                                                start=(k == 0), stop=(k == UC - 1),
                                            )
                                        nc.vector.tensor_add(
                                            st_new[:, m, :], ps, xpsb[:, m, j, :])
                                    st_prev = st_new
                                    nc.scalar.dma_start(
                                        out=osc_v[:, :, j, :], in_=st_new)
                                    for h in range(2):
                                        w = 2 * j + h
                                        nc.sync.dma_start(
                                            out=oacc[:, h, jg, :],
                                            in_=osc_flat[:, w * P:(w + 1) * P],
                                            transpose=True)
                                for h in range(2):
                                    nc.gpsimd.dma_start(
                                        out=out4[h * 16:(h + 1) * 16, :,
                                                 g * (K // 2):(g + 1) * (K // 2), :],
                                        in_=oacc[:, h, :, :],
                                    )

        with tc.tile_pool(name="repsp", bufs=1) as reps_p:
            rtile = reps_p.tile([1, 1], mybir.dt.int32)
            nc.sync.dma_start(out=rtile, in_=reps_in)
            reps_val = nc.values_load(rtile[0:1, 0:1])
            with tc.For_i(0, reps_val, 1) as _it:
                _body(_it)

    nc.compile()
    return nc


_NC_CACHE = {}


def _get_nc(reps=1):
    if "nc" not in _NC_CACHE:
        _NC_CACHE["nc"] = build_nc()
    return _NC_CACHE["nc"]


def _make_in_maps(sequences, kernel, recurrent_kernel, h0, reps=1):
    in_maps = []
    for c in range(NCORES):
        sl = slice(c * BC, (c + 1) * BC)
        in_maps.append({
            "seq": sequences[sl],
            "w": kernel,
            "r": recurrent_kernel,
            "h0": h0[sl],
            "reps": np.array([[reps]], dtype=np.int32),
        })
    return in_maps


def bench(inputs, reps):
    from concourse.bass_utils import run_bass_kernel_spmd
    nc = _get_nc()
    in_maps = _make_in_maps(
        np.ascontiguousarray(inputs["sequences"], dtype=np.float32),
        np.ascontiguousarray(inputs["kernel"], dtype=np.float32),
        np.ascontiguousarray(inputs["recurrent_kernel"], dtype=np.float32),
        np.ascontiguousarray(inputs["h0"], dtype=np.float32), reps)
    return run_bass_kernel_spmd(nc, in_maps, core_ids=list(range(NCORES)))


def kernel(sequences, kernel, recurrent_kernel, h0):
    from concourse.bass_utils import run_bass_kernel_spmd
    nc = _get_nc()
    sequences = np.ascontiguousarray(sequences, dtype=np.float32)
    kernel = np.ascontiguousarray(kernel, dtype=np.float32)
    recurrent_kernel = np.ascontiguousarray(recurrent_kernel, dtype=np.float32)
    h0 = np.ascontiguousarray(h0, dtype=np.float32)
    in_maps = _make_in_maps(sequences, kernel, recurrent_kernel, h0)
    res = run_bass_kernel_spmd(nc, in_maps, core_ids=list(range(NCORES)))
    return np.concatenate([res.results[c]["out"] for c in range(NCORES)], axis=0)


# ---------------------------------------------------------------- dev tools
def _numpy_model(seqs, W, R, h0):
    xp = seqs.reshape(-1, F) @ W
    xp = xp.reshape(seqs.shape[0], T, U)
    h = h0.copy()
    outs = np.zeros((seqs.shape[0], T, U), np.float32)
    for t in range(T):
        h = xp[:, t] + h @ R
        outs[:, t] = h
    return outs
